# revision 32
# baseline (speedup 1.0000x reference)
"""Causal multi-head attention (B=2, S=2048, D=2048, H=16) on 8 TRN2 cores.

Sharding: core c = (batch b = c//4, head-group r = c%4 -> heads 4r..4r+3).
Per core: project q/k/v for its 4 heads over all tokens (bf16 matmuls, fp32
PSUM), RoPE, exact-causal attention in transposed-score layout (scoresT
[keys, q] via lhsT=k_fm, rhs=q_fm; z[dh, q] via lhsT=v_tokmajor, rhs=e).
Softmax denominator is accumulated on the TensorEngine (ones-matmul) into
the second half of a [128,1024] PSUM tile shared with the z accumulation.
Output projection partials go through a per-phase bf16 ReduceScatter
across the 4 cores of each batch group.

Numerics: bf16 matmul inputs everywhere (fp32 PSUM accumulation), exp on
ACT (no max-subtraction; scores are O(1)), reciprocal in fp32.
"""
import sys

sys.path.insert(0, "/opt/trn_rl_repo")

from contextlib import ExitStack

import ml_dtypes
import numpy as np

import concourse.bass as bass  # noqa: F401  (bass must import before tile)
import concourse.mybir as mybir
import concourse.tile as tile
from concourse import bacc
from concourse.bass_utils import run_bass_kernel_spmd

dt = mybir.dt
BF16 = ml_dtypes.bfloat16
P = 128
D = 2048
N_HEAD = 16
DH = 128
HPC = 4            # heads per core
ROPE_BASE = 10000.0
GROUPS = [[0, 1, 2, 3], [4, 5, 6, 7]]


def _build(S: int):
    NP = S // 512  # token phases
    f32, bf = dt.float32, dt.bfloat16
    nc = bacc.Bacc(None, target_bir_lowering=False, num_devices=8)

    xT = nc.declare_dram_parameter("xT", [P, 16, S], bf, isOutput=False)
    wq = nc.declare_dram_parameter("wq", [P, 16, 512], bf, isOutput=False)
    wk = nc.declare_dram_parameter("wk", [P, 16, 512], bf, isOutput=False)
    wv = nc.declare_dram_parameter("wv", [P, 16, 512], bf, isOutput=False)
    wo = nc.declare_dram_parameter("wo", [P, HPC, 16, P], bf, isOutput=False)
    cosk = nc.declare_dram_parameter("cosk", [P, S], bf, isOutput=False)
    sink = nc.declare_dram_parameter("sink", [P, S], bf, isOutput=False)
    maskk = nc.declare_dram_parameter("maskk", [P, P], bf, isOutput=False)
    negtri = nc.declare_dram_parameter("negtri", [P, P], bf, isOutput=False)
    permm = nc.declare_dram_parameter("permm", [P, P], bf, isOutput=False)
    out_sh = nc.declare_dram_parameter("out_sh", [NP, 512, 512], bf,
                                       isOutput=True)

    rs_in = [nc.dram_tensor(f"rs_in{T}", [4 * 512, 512], bf)
             for T in range(NP)]
    rs_r = [t.rearrange("(mg mi p) s -> mg p mi s", p=P, mi=4) for t in rs_in]
    rs_out = [nc.dram_tensor(f"rs_out{T}", [512, 512], bf) for T in range(NP)]

    with tile.TileContext(nc) as tc, ExitStack() as ctx:
        const = ctx.enter_context(tc.tile_pool(name="const", bufs=1))
        kvres = ctx.enter_context(tc.tile_pool(name="kvres", bufs=1))
        xp = ctx.enter_context(tc.tile_pool(name="xp", bufs=2))
        qp = ctx.enter_context(tc.tile_pool(name="qp", bufs=3))
        rp = ctx.enter_context(tc.tile_pool(name="rp", bufs=4))
        tp = ctx.enter_context(tc.tile_pool(name="tp", bufs=9))
        ep = ctx.enter_context(tc.tile_pool(name="ep", bufs=8))
        bp = ctx.enter_context(tc.tile_pool(name="bp", bufs=2))
        dp = ctx.enter_context(tc.tile_pool(name="dp", bufs=3))
        zp = ctx.enter_context(tc.tile_pool(name="zp", bufs=2))
        op_ = ctx.enter_context(tc.tile_pool(name="op", bufs=2))
        pp = ctx.enter_context(tc.tile_pool(name="pp", bufs=2, space="PSUM"))
        sc = ctx.enter_context(tc.tile_pool(name="sc", bufs=2, space="PSUM"))
        zd = ctx.enter_context(tc.tile_pool(name="zd", bufs=2, space="PSUM"))

        wq_sb = const.tile([P, 16, 512], bf, name="wq_sb")
        wk_sb = const.tile([P, 16, 512], bf, name="wk_sb")
        wv_sb = const.tile([P, 16, 512], bf, name="wv_sb")
        wo_sb = const.tile([P, HPC, 16, P], bf, name="wo_sb")
        cos_sb = const.tile([P, S], bf, name="cos_sb")
        sin_sb = const.tile([P, S], bf, name="sin_sb")
        ident_sb = const.tile([P, P], bf, name="ident_sb")
        negtri_sb = const.tile([P, P], bf, name="negtri_sb")
        permm_sb = const.tile([P, P], bf, name="permm_sb")
        ones_sb = const.tile([P, P], bf, name="ones_sb")

        def load_consts():
            # wq/x phase-0 chunks are emitted by proj_phase(0) before this
            for ks in (slice(0, 4), slice(4, 8), slice(8, 12),
                       slice(12, 16)):
                nc.sync.dma_start(out=wk_sb[:, ks, :], in_=wk[:, ks, :])
            nc.sync.dma_start(out=cos_sb, in_=cosk[:, :])
            nc.sync.dma_start(out=sin_sb, in_=sink[:, :])
            nc.sync.dma_start(out=permm_sb, in_=permm[:, :])
            for half in range(2):
                ks = slice(8 * half, 8 * half + 8)
                nc.sync.dma_start(out=wv_sb[:, ks, :], in_=wv[:, ks, :])
            nc.sync.dma_start(out=wo_sb, in_=wo[:, :, :, :])
            nc.sync.dma_start(out=ident_sb, in_=maskk[:, :])
            nc.sync.dma_start(out=negtri_sb, in_=negtri[:, :])
            nc.vector.memset(ones_sb, 1.0)

        # persistent K (feature-major) and V (token-major) per 512-token phase
        k_sbs = [kvres.tile([P, HPC, 512], bf, tag=f"k_sb{T}", name=f"k_sb{T}")
                 for T in range(NP)]
        v_sbs = [kvres.tile([P, 4, 512], bf, tag=f"v_sb{T}", name=f"v_sb{T}")
                 for T in range(NP)]

        q_sbs = {}
        z_sbs = {}
        x_tiles = {}

        def load_x(T):
            if T in x_tiles or T >= NP:
                return
            tok = slice(512 * T, 512 * (T + 1))
            x_t = xp.tile([P, 16, 512], bf, tag="x_t", name=f"x_{T}")
            nc.sync.dma_start(out=x_t[:, 0:8, :], in_=xT[:, 0:8, tok])
            nc.sync.dma_start(out=x_t[:, 8:16, :], in_=xT[:, 8:16, tok])
            x_tiles[T] = x_t

        def proj_phase(T):
            tok = slice(512 * T, 512 * (T + 1))
            if T == 0:
                x_t = xp.tile([P, 16, 512], bf, tag="x_t", name=f"x_{T}")
                x_tiles[T] = x_t
                # interleave x and wq chunks so projection matmuls can start
                # as early as possible; everything else follows
                for ks in (slice(0, 2), slice(2, 4), slice(4, 8),
                           slice(8, 12), slice(12, 16)):
                    nc.sync.dma_start(out=x_t[:, ks, :], in_=xT[:, ks, tok])
                    nc.sync.dma_start(out=wq_sb[:, ks, :], in_=wq[:, ks, :])
                load_consts()
            else:
                load_x(T)
            x_t = x_tiles.pop(T)
            load_x(T + 1)   # prefetch next phase's activations

            # ---- Q / K projections with RoPE rotations staggered two
            # matmul-groups behind (PSUM evac overlaps the next group, and
            # the rope DVE work overlaps later groups instead of tailing)
            q_sb = qp.tile([P, HPC, 512], bf, tag="q_sb", name=f"q_sb{T}")
            q_sbs[T] = q_sb
            pending = []

            def proj_group(wt_sb, is_q, h):
                ps = pp.tile([P, 512], f32, tag="pp")
                for kd in range(16):
                    nc.tensor.matmul(ps[:],
                                     lhsT=wt_sb[:, kd, P * h:P * (h + 1)],
                                     rhs=x_t[:, kd, :],
                                     start=(kd == 0), stop=(kd == 15))
                t = tp.tile([P, 512], bf, tag="t")
                if is_q:   # fold the 1/sqrt(Dh) score scale into q
                    nc.scalar.mul(t[:], ps[:], float(DH) ** -0.5)
                else:
                    nc.scalar.copy(t[:], ps[:])
                pending.append((t, is_q, h))

            def v_group(tb):
                psv = pp.tile([P, 512], f32, tag="pp")
                for kd in range(16):
                    nc.tensor.matmul(psv[:],
                                     lhsT=x_t[:, kd, P * tb:P * (tb + 1)],
                                     rhs=wv_sb[:, kd, :],
                                     start=(kd == 0), stop=(kd == 15))
                nc.scalar.copy(v_sbs[T][:, tb, :], psv[:])

            def rot_head():
                t, is_q, h = pending.pop(0)
                ps2 = pp.tile([P, 512], f32, tag="pp")
                nc.tensor.matmul(ps2[:], lhsT=permm_sb[:], rhs=t[:],
                                 start=True, stop=True)
                u = rp.tile([P, 512], bf, tag="u")
                nc.vector.tensor_mul(u[:], t[:], cos_sb[:, tok])
                sw = rp.tile([P, 512], bf, tag="sw")
                nc.vector.tensor_mul(sw[:], ps2[:], sin_sb[:, tok])
                dst = q_sb[:, h, :] if is_q else k_sbs[T][:, h, :]
                nc.vector.tensor_add(dst, u[:], sw[:])

            work = [(proj_group, (wt_sb, is_q, h))
                    for wt_sb, is_q in ((wq_sb, True), (wk_sb, False))
                    for h in range(HPC)]
            work += [(v_group, (tb,)) for tb in range(4)]
            for i, (fn, args) in enumerate(work):
                fn(*args)
                # rotations trail ~5 groups behind: their DVE work overlaps
                # the second half of the projection groups + V projection
                if i >= 1 and pending and len(pending) + i >= 9:
                    rot_head()
            while pending:
                rot_head()

        def attn_phase(T):
            q_sb = q_sbs.pop(T)
            z_sb = zp.tile([P, HPC, 512], bf, tag="z_sb", name=f"z_sb{T}")
            nkb = 4 * T + 4
            # off-diagonal key blocks processed in pairs sharing one exp op;
            # diagonal blocks stay single (range-restricted + triangle bias)
            groups = [(2 * i, 2 * i + 1) for i in range(2 * T)]
            groups += [(kb,) for kb in range(4 * T, nkb)]
            for h in range(HPC):
                ps_z = zd.tile([P, 512], f32, tag="zd",
                               name=f"ps_z{T}_{h}")
                den = dp.tile([P, 512], bf, tag="den")
                for g in groups:
                    ps_s = sc.tile([P, 2, 512], f32, tag="sc")
                    e = ep.tile([P, 2, 512], bf, tag="e")
                    for idx, kb in enumerate(g):
                        j = kb - 4 * T
                        qlo = P * j if j > 0 else 0
                        qs = slice(qlo, 512)
                        diag = j >= 0
                        nc.tensor.matmul(
                            ps_s[:, idx, qs],
                            lhsT=k_sbs[kb // 4][:, h,
                                                P * (kb % 4):P * (kb % 4 + 1)],
                            rhs=q_sb[:, h, qs],
                            start=True, stop=not diag)
                        if diag:  # causal: add -1e9 upper triangle
                            nc.tensor.matmul(
                                ps_s[:, idx, qlo:qlo + P],
                                lhsT=ident_sb[:], rhs=negtri_sb[:],
                                start=False, stop=True, skip_group_check=True)
                    if len(g) == 2:
                        nc.scalar.activation(
                            e[:, :, :], ps_s[:, :, :],
                            mybir.ActivationFunctionType.Exp)
                    else:
                        j = g[0] - 4 * T
                        qlo = P * j if j > 0 else 0
                        nc.scalar.activation(
                            e[:, 0, qlo:], ps_s[:, 0, qlo:],
                            mybir.ActivationFunctionType.Exp)
                    for idx, kb in enumerate(g):
                        j = kb - 4 * T
                        qlo = P * j if j > 0 else 0
                        qs = slice(qlo, 512)
                        first, last = (kb == 0), (kb == nkb - 1)
                        nc.tensor.matmul(
                            ps_z[:, qs],
                            lhsT=v_sbs[kb // 4][:, kb % 4,
                                                P * h:P * (h + 1)],
                            rhs=e[:, idx, qs],
                            start=first, stop=last, skip_group_check=True)
                        if first:
                            nc.vector.tensor_copy(den[:], e[:, 0, :])
                        else:
                            nc.vector.tensor_add(den[:, qs], den[:, qs],
                                                 e[:, idx, qs])
                ps_f = sc.tile([P, 512], f32, tag="sc")
                nc.tensor.matmul(ps_f[:], lhsT=ones_sb[:], rhs=den[:],
                                 start=True, stop=True)
                bc = bp.tile([P, 512], f32, tag="bc")
                nc.vector.reciprocal(bc[:], ps_f[:])
                nc.vector.tensor_mul(z_sb[:, h, :], ps_z[:], bc[:])
            z_sbs[T] = z_sb

        def wo_phase(T):
            z_sb = z_sbs.pop(T)
            for mg in range(4):
                o_sb = op_.tile([P, 4, 512], bf, tag="o_sb")
                for mi in range(4):
                    m = 4 * mg + mi
                    ps_o = pp.tile([P, 512], f32, tag="pp")
                    for kd in range(HPC):
                        nc.tensor.matmul(ps_o[:],
                                         lhsT=wo_sb[:, kd, m, :],
                                         rhs=z_sb[:, kd, :],
                                         start=(kd == 0), stop=(kd == HPC - 1))
                    nc.vector.tensor_copy(o_sb[:, mi, :], ps_o[:])
                nc.sync.dma_start(out=rs_r[T][mg], in_=o_sb[:])
            nc.gpsimd.collective_compute(
                "ReduceScatter", mybir.AluOpType.add, replica_groups=GROUPS,
                ins=[rs_in[T][:, :]], outs=[rs_out[T][:, :]])
            nc.sync.dma_start(out=out_sh[T, :, :], in_=rs_out[T][:, :])

        for T in range(NP):
            proj_phase(T)
            if T >= 1:
                attn_phase(T - 1)
                wo_phase(T - 1)
        attn_phase(NP - 1)
        wo_phase(NP - 1)

    nc.compile()
    return nc


_BUILT = {}


def _get_built(S):
    if S not in _BUILT:
        _BUILT[S] = _build(S)
    return _BUILT[S]


def _bf16(x: np.ndarray) -> np.ndarray:
    return np.ascontiguousarray(x.astype(BF16))


def host_inputs(x, w_qkv, w_o):
    """Build the 8 per-core input maps from full inputs."""
    B, S, D_ = x.shape

    j = np.arange(0, DH, 2, dtype=np.float32) / DH          # (2j)/Dh, j=0..63
    inv_freq = (1.0 / (ROPE_BASE ** j)).astype(np.float32)  # [64]
    t = np.arange(S, dtype=np.float32)
    freqs = np.outer(inv_freq, t)                            # [64, S]
    emb = np.concatenate([freqs, freqs], axis=0)             # [128, S]
    cos_t = _bf16(np.cos(emb))
    sin_t = _bf16(np.sin(emb))
    # rot = R @ q (rotate_half incl. sign); matmul computes lhsT.T @ rhs,
    # so feed R.T: R[d, d+64] = -1 (d<64), R[d, d-64] = +1 (d>=64)
    permm_np = np.zeros((P, P), dtype=np.float32)
    for d_ in range(64):
        permm_np[d_ + 64, d_] = -1.0
        permm_np[d_, d_ + 64] = 1.0
    permm_np = _bf16(permm_np)

    k_idx = np.arange(P)[:, None]
    q_idx = np.arange(P)[None, :]
    mask_np = _bf16(np.eye(P, dtype=np.float32))             # identity lhsT
    negtri_np = _bf16((q_idx < k_idx).astype(np.float32) * -1e9)

    wqkvT = np.asarray(w_qkv, dtype=np.float32).T            # [D, 3D]
    woT_full = np.asarray(w_o, dtype=np.float32).T           # [D(in), D(out)]

    def wslice(r, n):
        ws = wqkvT[:, n * D + 512 * r:n * D + 512 * (r + 1)]  # [D, 512]
        return _bf16(ws.reshape(16, P, 512).transpose(1, 0, 2))

    xTb = [
        _bf16(np.ascontiguousarray(x[b].T).reshape(16, P, S).transpose(1, 0, 2))
        for b in range(B)
    ]

    in_maps = []
    for c in range(8):
        b, r = c // 4, c % 4
        woc = woT_full[512 * r:512 * (r + 1), :]              # [512, D]
        wo_np = _bf16(woc.reshape(HPC, P, 16, P).transpose(1, 0, 2, 3))
        in_maps.append({
            "xT": xTb[b],
            "wq": wslice(r, 0),
            "wk": wslice(r, 1),
            "wv": wslice(r, 2),
            "wo": wo_np,
            "cosk": cos_t, "sink": sin_t,
            "maskk": mask_np, "negtri": negtri_np, "permm": permm_np,
        })
    return in_maps


def assemble(results, B, S):
    NP = S // 512
    out = np.empty((B, S, D), dtype=np.float32)
    for c in range(8):
        b, r = c // 4, c % 4
        sh = np.asarray(results[c]["out_sh"]).astype(np.float32)
        for T in range(NP):
            out[b, 512 * T:512 * (T + 1), 512 * r:512 * (r + 1)] = sh[T].T
    return out


def kernel(x, w_qkv, w_o, _trace=False):
    x = np.asarray(x, dtype=np.float32)
    w_qkv = np.asarray(w_qkv, dtype=np.float32)
    w_o = np.asarray(w_o, dtype=np.float32)
    B, S, _ = x.shape
    nc = _get_built(S)
    in_maps = host_inputs(x, w_qkv, w_o)

    def _run():
        try:
            return run_bass_kernel_spmd(nc, in_maps, list(range(8)),
                                        trace=_trace)
        except ModuleNotFoundError:
            return run_bass_kernel_spmd(nc, in_maps, list(range(8)))

    try:
        res = _run()
    except Exception:
        res = _run()  # transient runtime/readback errors: retry once
    out = assemble(res.results, B, S)
    if _trace:
        return out, res
    return out


# revision 33
# speedup vs baseline: 1.0018x; 1.0018x over previous
"""Causal multi-head attention (B=2, S=2048, D=2048, H=16) on 8 TRN2 cores.

Sharding: core c = (batch b = c//4, head-group r = c%4 -> heads 4r..4r+3).
Per core: project q/k/v for its 4 heads over all tokens (bf16 matmuls, fp32
PSUM), RoPE, exact-causal attention in transposed-score layout (scoresT
[keys, q] via lhsT=k_fm, rhs=q_fm; z[dh, q] via lhsT=v_tokmajor, rhs=e).
Softmax denominator is accumulated on the TensorEngine (ones-matmul) into
the second half of a [128,1024] PSUM tile shared with the z accumulation.
Output projection partials go through a per-phase bf16 ReduceScatter
across the 4 cores of each batch group.

Numerics: bf16 matmul inputs everywhere (fp32 PSUM accumulation), exp on
ACT (no max-subtraction; scores are O(1)), reciprocal in fp32.
"""
import sys

sys.path.insert(0, "/opt/trn_rl_repo")

from contextlib import ExitStack

import ml_dtypes
import numpy as np

import concourse.bass as bass  # noqa: F401  (bass must import before tile)
import concourse.mybir as mybir
import concourse.tile as tile
from concourse import bacc
from concourse.bass_utils import run_bass_kernel_spmd

dt = mybir.dt
BF16 = ml_dtypes.bfloat16
P = 128
D = 2048
N_HEAD = 16
DH = 128
HPC = 4            # heads per core
ROPE_BASE = 10000.0
GROUPS = [[0, 1, 2, 3], [4, 5, 6, 7]]


def _build(S: int):
    NP = S // 512  # token phases
    f32, bf = dt.float32, dt.bfloat16
    nc = bacc.Bacc(None, target_bir_lowering=False, num_devices=8)

    xT = nc.declare_dram_parameter("xT", [P, 16, S], bf, isOutput=False)
    wq = nc.declare_dram_parameter("wq", [P, 16, 512], bf, isOutput=False)
    wk = nc.declare_dram_parameter("wk", [P, 16, 512], bf, isOutput=False)
    wv = nc.declare_dram_parameter("wv", [P, 16, 512], bf, isOutput=False)
    wo = nc.declare_dram_parameter("wo", [P, HPC, 16, P], bf, isOutput=False)
    cosk = nc.declare_dram_parameter("cosk", [P, S], bf, isOutput=False)
    sink = nc.declare_dram_parameter("sink", [P, S], bf, isOutput=False)
    maskk = nc.declare_dram_parameter("maskk", [P, P], bf, isOutput=False)
    negtri = nc.declare_dram_parameter("negtri", [P, P], bf, isOutput=False)
    permm = nc.declare_dram_parameter("permm", [P, P], bf, isOutput=False)
    out_sh = nc.declare_dram_parameter("out_sh", [NP, 512, 512], bf,
                                       isOutput=True)

    rs_in = [nc.dram_tensor(f"rs_in{T}", [4 * 512, 512], bf)
             for T in range(NP)]
    rs_r = [t.rearrange("(mg mi p) s -> mg p mi s", p=P, mi=4) for t in rs_in]
    rs_out = [nc.dram_tensor(f"rs_out{T}", [512, 512], bf) for T in range(NP)]

    with tile.TileContext(nc) as tc, ExitStack() as ctx:
        const = ctx.enter_context(tc.tile_pool(name="const", bufs=1))
        kvres = ctx.enter_context(tc.tile_pool(name="kvres", bufs=1))
        xp = ctx.enter_context(tc.tile_pool(name="xp", bufs=2))
        qp = ctx.enter_context(tc.tile_pool(name="qp", bufs=3))
        rp = ctx.enter_context(tc.tile_pool(name="rp", bufs=4))
        tp = ctx.enter_context(tc.tile_pool(name="tp", bufs=9))
        ep = ctx.enter_context(tc.tile_pool(name="ep", bufs=8))
        bp = ctx.enter_context(tc.tile_pool(name="bp", bufs=2))
        dp = ctx.enter_context(tc.tile_pool(name="dp", bufs=3))
        zp = ctx.enter_context(tc.tile_pool(name="zp", bufs=2))
        op_ = ctx.enter_context(tc.tile_pool(name="op", bufs=2))
        pp = ctx.enter_context(tc.tile_pool(name="pp", bufs=2, space="PSUM"))
        sc = ctx.enter_context(tc.tile_pool(name="sc", bufs=2, space="PSUM"))
        zd = ctx.enter_context(tc.tile_pool(name="zd", bufs=2, space="PSUM"))

        wq_sb = const.tile([P, 16, 512], bf, name="wq_sb")
        wk_sb = const.tile([P, 16, 512], bf, name="wk_sb")
        wv_sb = const.tile([P, 16, 512], bf, name="wv_sb")
        wo_sb = const.tile([P, HPC, 16, P], bf, name="wo_sb")
        cos_sb = const.tile([P, S], bf, name="cos_sb")
        sin_sb = const.tile([P, S], bf, name="sin_sb")
        ident_sb = const.tile([P, P], bf, name="ident_sb")
        negtri_sb = const.tile([P, P], bf, name="negtri_sb")
        permm_sb = const.tile([P, P], bf, name="permm_sb")
        ones_sb = const.tile([P, P], bf, name="ones_sb")

        def load_consts():
            # wq/x phase-0 chunks are emitted by proj_phase(0) before this
            for ks in (slice(0, 4), slice(4, 8), slice(8, 12),
                       slice(12, 16)):
                nc.sync.dma_start(out=wk_sb[:, ks, :], in_=wk[:, ks, :])
            nc.sync.dma_start(out=cos_sb, in_=cosk[:, :])
            nc.sync.dma_start(out=sin_sb, in_=sink[:, :])
            nc.sync.dma_start(out=permm_sb, in_=permm[:, :])
            for half in range(2):
                ks = slice(8 * half, 8 * half + 8)
                nc.sync.dma_start(out=wv_sb[:, ks, :], in_=wv[:, ks, :])
            nc.sync.dma_start(out=wo_sb, in_=wo[:, :, :, :])
            nc.sync.dma_start(out=ident_sb, in_=maskk[:, :])
            nc.sync.dma_start(out=negtri_sb, in_=negtri[:, :])
            nc.vector.memset(ones_sb, 1.0)

        # persistent K (feature-major) and V (token-major) per 512-token phase
        k_sbs = [kvres.tile([P, HPC, 512], bf, tag=f"k_sb{T}", name=f"k_sb{T}")
                 for T in range(NP)]
        v_sbs = [kvres.tile([P, 4, 512], bf, tag=f"v_sb{T}", name=f"v_sb{T}")
                 for T in range(NP)]

        q_sbs = {}
        z_sbs = {}
        x_tiles = {}

        def load_x(T):
            if T in x_tiles or T >= NP:
                return
            tok = slice(512 * T, 512 * (T + 1))
            x_t = xp.tile([P, 16, 512], bf, tag="x_t", name=f"x_{T}")
            nc.sync.dma_start(out=x_t[:, 0:8, :], in_=xT[:, 0:8, tok])
            nc.sync.dma_start(out=x_t[:, 8:16, :], in_=xT[:, 8:16, tok])
            x_tiles[T] = x_t

        def proj_phase(T):
            tok = slice(512 * T, 512 * (T + 1))
            if T == 0:
                x_t = xp.tile([P, 16, 512], bf, tag="x_t", name=f"x_{T}")
                x_tiles[T] = x_t
                # interleave x and wq chunks so projection matmuls can start
                # as early as possible; everything else follows
                for ks in (slice(0, 2), slice(2, 4), slice(4, 8),
                           slice(8, 12), slice(12, 16)):
                    nc.sync.dma_start(out=x_t[:, ks, :], in_=xT[:, ks, tok])
                    nc.sync.dma_start(out=wq_sb[:, ks, :], in_=wq[:, ks, :])
                load_consts()
            else:
                load_x(T)
            x_t = x_tiles.pop(T)
            load_x(T + 1)   # prefetch next phase's activations

            # ---- Q / K projections with RoPE rotations staggered two
            # matmul-groups behind (PSUM evac overlaps the next group, and
            # the rope DVE work overlaps later groups instead of tailing)
            q_sb = qp.tile([P, HPC, 512], bf, tag="q_sb", name=f"q_sb{T}")
            q_sbs[T] = q_sb
            pending = []

            def proj_group(wt_sb, is_q, h):
                ps = pp.tile([P, 512], f32, tag="pp")
                for kd in range(16):
                    nc.tensor.matmul(ps[:],
                                     lhsT=wt_sb[:, kd, P * h:P * (h + 1)],
                                     rhs=x_t[:, kd, :],
                                     start=(kd == 0), stop=(kd == 15))
                t = tp.tile([P, 512], bf, tag="t")
                if is_q:   # fold the 1/sqrt(Dh) score scale into q
                    nc.scalar.mul(t[:], ps[:], float(DH) ** -0.5)
                else:
                    nc.scalar.copy(t[:], ps[:])
                pending.append((t, is_q, h))

            def v_group(tb):
                psv = pp.tile([P, 512], f32, tag="pp")
                for kd in range(16):
                    nc.tensor.matmul(psv[:],
                                     lhsT=x_t[:, kd, P * tb:P * (tb + 1)],
                                     rhs=wv_sb[:, kd, :],
                                     start=(kd == 0), stop=(kd == 15))
                nc.scalar.copy(v_sbs[T][:, tb, :], psv[:])

            def rot_head():
                t, is_q, h = pending.pop(0)
                ps2 = pp.tile([P, 512], f32, tag="pp")
                nc.tensor.matmul(ps2[:], lhsT=permm_sb[:], rhs=t[:],
                                 start=True, stop=True)
                u = rp.tile([P, 512], bf, tag="u")
                nc.vector.tensor_mul(u[:], t[:], cos_sb[:, tok])
                sw = rp.tile([P, 512], bf, tag="sw")
                nc.vector.tensor_mul(sw[:], ps2[:], sin_sb[:, tok])
                dst = q_sb[:, h, :] if is_q else k_sbs[T][:, h, :]
                nc.vector.tensor_add(dst, u[:], sw[:])

            work = [(proj_group, (wt_sb, is_q, h))
                    for wt_sb, is_q in ((wq_sb, True), (wk_sb, False))
                    for h in range(HPC)]
            work += [(v_group, (tb,)) for tb in range(4)]
            for i, (fn, args) in enumerate(work):
                fn(*args)
                # rotations trail ~5 groups behind: their DVE work overlaps
                # the second half of the projection groups + V projection
                if i >= 1 and pending and len(pending) + i >= 9:
                    rot_head()
            while pending:
                rot_head()

        def attn_phase(T):
            q_sb = q_sbs.pop(T)
            z_sb = zp.tile([P, HPC, 512], bf, tag="z_sb", name=f"z_sb{T}")
            nkb = 4 * T + 4
            # off-diagonal key blocks processed in pairs sharing one exp op;
            # diagonal blocks stay single (range-restricted + triangle bias)
            groups = [(2 * i, 2 * i + 1) for i in range(2 * T)]
            groups += [(kb,) for kb in range(4 * T, nkb)]
            for h in range(HPC):
                ps_z = zd.tile([P, 512], f32, tag="zd",
                               name=f"ps_z{T}_{h}")
                den = dp.tile([P, 512], bf, tag="den")
                for g in groups:
                    ps_s = sc.tile([P, 2, 512], f32, tag="sc")
                    e = ep.tile([P, 2, 512], bf, tag="e")
                    for idx, kb in enumerate(g):
                        j = kb - 4 * T
                        qlo = P * j if j > 0 else 0
                        qs = slice(qlo, 512)
                        diag = j >= 0
                        nc.tensor.matmul(
                            ps_s[:, idx, qs],
                            lhsT=k_sbs[kb // 4][:, h,
                                                P * (kb % 4):P * (kb % 4 + 1)],
                            rhs=q_sb[:, h, qs],
                            start=True, stop=not diag)
                        if diag:  # causal: add -1e9 upper triangle
                            nc.tensor.matmul(
                                ps_s[:, idx, qlo:qlo + P],
                                lhsT=ident_sb[:], rhs=negtri_sb[:],
                                start=False, stop=True, skip_group_check=True)
                    if len(g) == 2:
                        nc.scalar.activation(
                            e[:, :, :], ps_s[:, :, :],
                            mybir.ActivationFunctionType.Exp)
                    else:
                        j = g[0] - 4 * T
                        qlo = P * j if j > 0 else 0
                        nc.scalar.activation(
                            e[:, 0, qlo:], ps_s[:, 0, qlo:],
                            mybir.ActivationFunctionType.Exp)
                    for idx, kb in enumerate(g):
                        j = kb - 4 * T
                        qlo = P * j if j > 0 else 0
                        qs = slice(qlo, 512)
                        first, last = (kb == 0), (kb == nkb - 1)
                        nc.tensor.matmul(
                            ps_z[:, qs],
                            lhsT=v_sbs[kb // 4][:, kb % 4,
                                                P * h:P * (h + 1)],
                            rhs=e[:, idx, qs],
                            start=first, stop=last, skip_group_check=True)
                        if first:
                            nc.vector.tensor_copy(den[:], e[:, 0, :])
                        else:
                            nc.vector.tensor_add(den[:, qs], den[:, qs],
                                                 e[:, idx, qs])
                ps_f = sc.tile([P, 512], f32, tag="sc")
                nc.tensor.matmul(ps_f[:], lhsT=ones_sb[:], rhs=den[:],
                                 start=True, stop=True)
                bc = bp.tile([P, 512], f32, tag="bc")
                nc.vector.reciprocal(bc[:], ps_f[:])
                nc.vector.tensor_mul(z_sb[:, h, :], ps_z[:], bc[:])
            z_sbs[T] = z_sb

        def wo_phase(T):
            z_sb = z_sbs.pop(T)
            last = (T == NP - 1)
            for mg in range(4):
                o_sb = op_.tile([P, 4, 512], bf, tag="o_sb")
                if last:
                    # tail: run on the attention score banks (free once the
                    # last z-chain drains) in m-pairs, one batched ACT evac
                    # per pair while the DVE finishes the z normalizations
                    for half in range(2):
                        ps_o = sc.tile([P, 2, 512], f32, tag="sc")
                        for mi2 in range(2):
                            m = 4 * mg + 2 * half + mi2
                            for kd in range(HPC):
                                nc.tensor.matmul(
                                    ps_o[:, mi2, :],
                                    lhsT=wo_sb[:, kd, m, :],
                                    rhs=z_sb[:, kd, :],
                                    start=(kd == 0), stop=(kd == HPC - 1))
                        nc.scalar.copy(
                            o_sb[:, 2 * half:2 * half + 2, :], ps_o[:, :, :])
                else:
                    for mi in range(4):
                        m = 4 * mg + mi
                        ps_o = pp.tile([P, 512], f32, tag="pp")
                        for kd in range(HPC):
                            nc.tensor.matmul(
                                ps_o[:],
                                lhsT=wo_sb[:, kd, m, :],
                                rhs=z_sb[:, kd, :],
                                start=(kd == 0), stop=(kd == HPC - 1))
                        nc.vector.tensor_copy(o_sb[:, mi, :], ps_o[:])
                nc.sync.dma_start(out=rs_r[T][mg], in_=o_sb[:])
            nc.gpsimd.collective_compute(
                "ReduceScatter", mybir.AluOpType.add, replica_groups=GROUPS,
                ins=[rs_in[T][:, :]], outs=[rs_out[T][:, :]])
            nc.sync.dma_start(out=out_sh[T, :, :], in_=rs_out[T][:, :])

        for T in range(NP):
            proj_phase(T)
            if T >= 1:
                attn_phase(T - 1)
                wo_phase(T - 1)
        attn_phase(NP - 1)
        wo_phase(NP - 1)

    nc.compile()
    return nc


_BUILT = {}


def _get_built(S):
    if S not in _BUILT:
        _BUILT[S] = _build(S)
    return _BUILT[S]


def _bf16(x: np.ndarray) -> np.ndarray:
    return np.ascontiguousarray(x.astype(BF16))


def host_inputs(x, w_qkv, w_o):
    """Build the 8 per-core input maps from full inputs."""
    B, S, D_ = x.shape

    j = np.arange(0, DH, 2, dtype=np.float32) / DH          # (2j)/Dh, j=0..63
    inv_freq = (1.0 / (ROPE_BASE ** j)).astype(np.float32)  # [64]
    t = np.arange(S, dtype=np.float32)
    freqs = np.outer(inv_freq, t)                            # [64, S]
    emb = np.concatenate([freqs, freqs], axis=0)             # [128, S]
    cos_t = _bf16(np.cos(emb))
    sin_t = _bf16(np.sin(emb))
    # rot = R @ q (rotate_half incl. sign); matmul computes lhsT.T @ rhs,
    # so feed R.T: R[d, d+64] = -1 (d<64), R[d, d-64] = +1 (d>=64)
    permm_np = np.zeros((P, P), dtype=np.float32)
    for d_ in range(64):
        permm_np[d_ + 64, d_] = -1.0
        permm_np[d_, d_ + 64] = 1.0
    permm_np = _bf16(permm_np)

    k_idx = np.arange(P)[:, None]
    q_idx = np.arange(P)[None, :]
    mask_np = _bf16(np.eye(P, dtype=np.float32))             # identity lhsT
    negtri_np = _bf16((q_idx < k_idx).astype(np.float32) * -1e9)

    wqkvT = np.asarray(w_qkv, dtype=np.float32).T            # [D, 3D]
    woT_full = np.asarray(w_o, dtype=np.float32).T           # [D(in), D(out)]

    def wslice(r, n):
        ws = wqkvT[:, n * D + 512 * r:n * D + 512 * (r + 1)]  # [D, 512]
        return _bf16(ws.reshape(16, P, 512).transpose(1, 0, 2))

    xTb = [
        _bf16(np.ascontiguousarray(x[b].T).reshape(16, P, S).transpose(1, 0, 2))
        for b in range(B)
    ]

    in_maps = []
    for c in range(8):
        b, r = c // 4, c % 4
        woc = woT_full[512 * r:512 * (r + 1), :]              # [512, D]
        wo_np = _bf16(woc.reshape(HPC, P, 16, P).transpose(1, 0, 2, 3))
        in_maps.append({
            "xT": xTb[b],
            "wq": wslice(r, 0),
            "wk": wslice(r, 1),
            "wv": wslice(r, 2),
            "wo": wo_np,
            "cosk": cos_t, "sink": sin_t,
            "maskk": mask_np, "negtri": negtri_np, "permm": permm_np,
        })
    return in_maps


def assemble(results, B, S):
    NP = S // 512
    out = np.empty((B, S, D), dtype=np.float32)
    for c in range(8):
        b, r = c // 4, c % 4
        sh = np.asarray(results[c]["out_sh"]).astype(np.float32)
        for T in range(NP):
            out[b, 512 * T:512 * (T + 1), 512 * r:512 * (r + 1)] = sh[T].T
    return out


def kernel(x, w_qkv, w_o, _trace=False):
    x = np.asarray(x, dtype=np.float32)
    w_qkv = np.asarray(w_qkv, dtype=np.float32)
    w_o = np.asarray(w_o, dtype=np.float32)
    B, S, _ = x.shape
    nc = _get_built(S)
    in_maps = host_inputs(x, w_qkv, w_o)

    def _run():
        try:
            return run_bass_kernel_spmd(nc, in_maps, list(range(8)),
                                        trace=_trace)
        except ModuleNotFoundError:
            return run_bass_kernel_spmd(nc, in_maps, list(range(8)))

    try:
        res = _run()
    except Exception:
        res = _run()  # transient runtime/readback errors: retry once
    out = assemble(res.results, B, S)
    if _trace:
        return out, res
    return out
